# revision 10
# baseline (speedup 1.0000x reference)
"""Trainium2 Bass kernel for nn_GroupPointEncoder.

Reference computation (G=4, B=8, N=2048, F=128):
  std = 2 or 4 per point by label class
  coords = [point_coord, (point_coord + noise*std)[1:]]           # [G,B,N,3]
  normed = (coords - low) / (high - low)
  pe     = interleaved sin/cos embedding, (y,x,z) order            # [G,B,N,384]
  h      = relu(pe @ W1.T + b1)                                    # [G,B,N,512]
  pos    = h @ W2.T + b2                                           # [G,B,N,256]
  query  = label_weight[labels] + pos
  out    = concat([query_pos, query], -1).reshape(G*B, N, 512)

Sharding: data-parallel over the G*B=32 (g,b) pairs, 4 per core, 8 cores.
Each core computes its 4*2048=8192 points' `query` half on device; the
query_pos half is a passthrough assembled on the host.

Device layout (feature-major, fp16 matmul path; all matmuls 1 cycle/row):
  pe partition rows per coord chunk: p<36 sin(s_{28+p} x), 36<=p<72
  cos(s_{p-8} x)  (cold rows, on-device ACT Sin with per-partition
  scale/bias; |s x| + pi/2 stays inside the Sin table domain), then 56
  "hot" rows (freqs 0..27) whose args can leave [-pi,pi]; their sin/cos
  values are computed exactly on the host and DMA'd straight into the
  pe tile.  bc is the host-replicated coordinate tile (no on-device
  broadcast, no range reduction -> GpSimd idle).
  h[128,4,T] = relu(W1p @ pe + b1)    12 fp16 matmuls + ACT/DVE relu
  q[128,2,T] = W2 @ h  (+ label row add at PSUM drain on DVE)
  Point-tiles are processed in PAIRS sharing each stationary weight
  block (halves LDWEIGHTS), and the PE queue is software-pipelined:
  s4(g), s5(g-1), s4(g+1), ... so the tensor engine never idles.
"""
import sys
import math

sys.path.insert(0, "/opt/trn_rl_repo")

import numpy as np
from contextlib import ExitStack

import concourse.bass as bass
import concourse.tile as tile
from concourse import bacc, mybir
from concourse.bass_utils import run_bass_kernel_spmd

# problem constants (hardcoded per contract)
G, B, N, F = 4, 8, 2048, 128
NCORES = 8
BPC = B * G // NCORES          # 4 (g,b) pairs per core
NPTS = BPC * N                 # 8192 points per core
T = 512                        # points per tile
NT = NPTS // T                 # 16 tiles
NG = NT // 2                   # 8 tile-pairs (weight-block reuse groups)
HOT = 28                       # freqs 0..27 need exact (host) reduction
COLD = 128 - 2 * HOT           # 72 cold rows (sin 36 + cos 36, freqs 28..63)
TWO_PI = 2.0 * math.pi
F32 = mybir.dt.float32
F16 = mybir.dt.float16

_CACHE = {}


def _build_program():
    nc = bacc.Bacc("TRN2", target_bir_lowering=False, debug=False, num_devices=NCORES)

    bc_d = nc.dram_tensor("bc", [NT, COLD, 3, T], F16, kind="ExternalInput").ap()
    peh_d = nc.dram_tensor("peh", [NT, 2 * HOT, 3, T], F16, kind="ExternalInput").ap()
    lab_d = nc.dram_tensor("lab", [NT, 128, 2, T], F16, kind="ExternalInput").ap()
    w1t_d = nc.dram_tensor("w1t", [3, 128, 512], F16, kind="ExternalInput").ap()
    w2t_d = nc.dram_tensor("w2t", [4, 128, 256], F16, kind="ExternalInput").ap()
    svec_d = nc.dram_tensor("svec", [128, 1], F32, kind="ExternalInput").ap()
    bvec_d = nc.dram_tensor("bvec", [128, 1], F32, kind="ExternalInput").ap()
    b1c_d = nc.dram_tensor("b1c", [128, 4], F32, kind="ExternalInput").ap()
    q_d = nc.dram_tensor("q", [NT, 256, T], F16, kind="ExternalOutput").ap()

    with tile.TileContext(nc) as tc, ExitStack() as ctx:
        cpool = ctx.enter_context(tc.tile_pool(name="consts", bufs=1))
        wpool = ctx.enter_context(tc.tile_pool(name="weights", bufs=1))
        io = ctx.enter_context(tc.tile_pool(name="io", bufs=8))
        labpool = ctx.enter_context(tc.tile_pool(name="lab", bufs=8))
        pepool = ctx.enter_context(tc.tile_pool(name="pe", bufs=6))
        hpool = ctx.enter_context(tc.tile_pool(name="h", bufs=4))
        qpool = ctx.enter_context(tc.tile_pool(name="qs", bufs=8))
        psum_h = ctx.enter_context(tc.tile_pool(name="ph", bufs=4, space="PSUM"))
        psum_q = ctx.enter_context(tc.tile_pool(name="pq", bufs=4, space="PSUM"))

        svec = cpool.tile([128, 1], F32)
        nc.sync.dma_start(svec[:], svec_d[:])
        bvec = cpool.tile([128, 1], F32)
        nc.sync.dma_start(bvec[:], bvec_d[:])
        b1c = cpool.tile([128, 4], F32)
        nc.sync.dma_start(b1c[:], b1c_d[:])

        w1t = []
        for k in range(3):
            w = wpool.tile([128, 512], F16, name=f"w1t{k}", tag=f"w1t{k}")
            nc.sync.dma_start(w[:], w1t_d[k])
            w1t.append(w)
        w2t = []
        for k in range(4):
            w = wpool.tile([128, 256], F16, name=f"w2t{k}", tag=f"w2t{k}")
            nc.sync.dma_start(w[:], w2t_d[k])
            w2t.append(w)

        warm = cpool.tile([128, 512], F16)
        nc.vector.memset(warm[:], 0.0)
        wsin = cpool.tile([128, 1], F16)
        nc.scalar.activation(
            wsin[:], warm[:, 0:1], mybir.ActivationFunctionType.Sin
        )
        wpsum = psum_q.tile([128, T], F32, name="wpsum", tag="qp")
        for _ in range(8):
            nc.tensor.matmul(wpsum[:], warm[:, 0:128], warm[:], start=True, stop=True)

        pend5 = []   # [(h_a, h_b, lab_a, lab_b, ta)] awaiting stage-5

        def emit_stage5_and_drain(h2, lab2, ta):
            # stage 5 for a tile group; each W2 block loaded once per group
            for mp in range(2):
                qps = []
                for p in range(len(h2)):
                    qp = psum_q.tile([128, T], F32, tag="qp")
                    qps.append(qp)
                for k in range(4):
                    for p in range(len(h2)):
                        nc.tensor.matmul(
                            qps[p][:],
                            w2t[k][:, mp * 128 : (mp + 1) * 128],
                            h2[p][:, k, :],
                            start=(k == 0),
                            stop=(k == 3),
                        )
                # drain on DVE with the label-row add fused in
                for p in range(len(h2)):
                    qs = qpool.tile([128, T], F16, tag="qs")
                    nc.vector.tensor_add(qs[:], qps[p][:], lab2[p][:, mp, :])
                    nc.sync.dma_start(
                        q_d[ta + p, mp * 128 : (mp + 1) * 128, :], qs[:]
                    )

        groups = [(0, 1)] + [(t, 2) for t in range(1, NT - 1, 2)] + [(NT - 1, 1)]

        def emit_inputs(ta, width):
            # input DMAs + cold-row Sin for each tile of a group
            pe2, lab2 = [], []
            for p in range(width):
                t = ta + p
                bc_t = io.tile([COLD, 3, T], F16, tag="bc_t")
                nc.gpsimd.dma_start(bc_t[:], bc_d[t])
                lab_t = labpool.tile([128, 2, T], F16, tag="lab_t")
                nc.sync.dma_start(lab_t[:], lab_d[t])
                pe_t = pepool.tile([128, 3, T], F16, tag="pe_t")
                nc.gpsimd.dma_start(pe_t[COLD:128, :, :], peh_d[t])
                nc.scalar.activation(
                    pe_t[0:COLD, :, :],
                    bc_t[:],
                    mybir.ActivationFunctionType.Sin,
                    bias=bvec[0:COLD, :],
                    scale=svec[0:COLD, :],
                )
                pe2.append(pe_t)
                lab2.append(lab_t)
            return pe2, lab2

        pending_in = [emit_inputs(*groups[0])]
        for gi, (ta, width) in enumerate(groups):
            # keep the ACT sin stream one group ahead of this group's relus
            if gi + 1 < len(groups):
                pending_in.append(emit_inputs(*groups[gi + 1]))
            pe2, lab2 = pending_in.pop(0)

            # ---- stage 4: each W1 block loaded once, used for both tiles
            h2 = [
                hpool.tile([128, 4, T], F16, name=f"h{p}", tag=f"h{p}")
                for p in range(width)
            ]
            for m in range(4):
                hps = [
                    psum_h.tile([128, T], F32, name=f"hp{p}", tag="hp")
                    for p in range(width)
                ]
                for k in range(3):
                    for p in range(width):
                        nc.tensor.matmul(
                            hps[p][:],
                            w1t[k][:, m * 128 : (m + 1) * 128],
                            pe2[p][:, k, :],
                            start=(k == 0),
                            stop=(k == 2),
                        )
                # relu + bias: alternate ACT / DVE to balance engine load
                for p in range(width):
                    if (m + p) % 2 == 0:
                        nc.scalar.activation(
                            h2[p][:, m, :],
                            hps[p][:],
                            mybir.ActivationFunctionType.Relu,
                            bias=b1c[:, m : m + 1],
                        )
                    else:
                        nc.vector.tensor_scalar(
                            h2[p][:, m, :],
                            hps[p][:],
                            b1c[:, m : m + 1],
                            0.0,
                            op0=mybir.AluOpType.add,
                            op1=mybir.AluOpType.max,
                        )

            # ---- stage 5 of the previous pair (keeps PE stream gapless)
            if pend5:
                emit_stage5_and_drain(*pend5.pop())
            pend5.append((h2, lab2, ta))

        while pend5:
            emit_stage5_and_drain(*pend5.pop())

    nc.compile()
    return nc


def _host_prep(point_coord, labels, pc_range, noise, label_weight, W1, b1, W2, b2):
    """Build the per-core input maps (host-side sharding + weight prep)."""
    pc32 = np.asarray(point_coord, np.float32)
    lab = np.asarray(labels, np.int64)
    noi = np.asarray(noise, np.float32)
    rng = np.asarray(pc_range, np.float32)

    small = (lab == 0) | (lab >= 6)
    std = np.where(small, 2.0, 4.0).astype(np.float32)            # [B,N]
    coords = pc32[None] + noi * std[None, :, :, None]             # [G,B,N,3]
    coords[0] = pc32                                              # group 0 originals
    low, high = rng[:3], rng[3:]
    pcs = (coords - low) / (high - low) * np.float32(TWO_PI)      # [G,B,N,3]
    pcs = pcs[..., [1, 0, 2]]   # reference concatenates pe in (y,x,z) order

    # partition layout (C=COLD//2=36, H=HOT=28):
    #   p<C sin k=HOT+p, C<=p<2C cos k=HOT+(p-C)  (cold, on-device Sin)
    #   2C<=p<2C+H sin k=p-2C, then cos k=p-2C-H  (hot, host-computed)
    k64 = np.arange(64, dtype=np.float64)
    s64 = 10000.0 ** (-k64 / 64.0)
    C = COLD // 2
    fmap = np.empty(128, np.int64)
    svec = np.zeros(128, np.float64)
    bvec = np.zeros(128, np.float64)
    fmap[0:C] = 2 * (HOT + np.arange(C))
    fmap[C : 2 * C] = 2 * (HOT + np.arange(C)) + 1
    fmap[2 * C : 2 * C + HOT] = 2 * np.arange(HOT)
    fmap[2 * C + HOT : 128] = 2 * np.arange(HOT) + 1
    svec[0:C] = s64[HOT:]
    svec[C : 2 * C] = s64[HOT:]
    bvec[C : 2 * C] = math.pi / 2
    perm = (np.arange(3)[:, None] * 128 + fmap[None, :]).reshape(-1)

    w1p = np.ascontiguousarray(np.asarray(W1, np.float32)[:, perm].T)  # [384,512]
    w2t = np.ascontiguousarray(np.asarray(W2, np.float32).T)           # [512,256]
    lwb = np.asarray(label_weight, np.float32) + np.asarray(b2, np.float32)[None]
    lemb = lwb[lab].astype(np.float16)                                 # [B,N,256]
    b1c = np.ascontiguousarray(np.asarray(b1, np.float32).reshape(4, 128).T)

    shared = {
        "w1t": w1p.reshape(3, 128, 512).astype(np.float16),
        "w2t": w2t.reshape(4, 128, 256).astype(np.float16),
        "svec": np.ascontiguousarray(svec.reshape(128, 1)).astype(np.float32),
        "bvec": np.ascontiguousarray(bvec.reshape(128, 1)).astype(np.float32),
        "b1c": b1c,
    }
    sh = s64[:HOT].astype(np.float32)

    in_maps = []
    for core in range(NCORES):
        g = core // 2
        b0 = 4 * (core % 2)
        # [4b, N, 3] -> [3, NT, T]
        x3 = pcs[g, b0 : b0 + 4].reshape(NPTS, 3).T.reshape(3, NT, T)
        bcc = np.ascontiguousarray(
            np.broadcast_to(x3[None], (COLD, 3, NT, T)).transpose(2, 0, 1, 3)
        ).astype(np.float16)                                      # [NT, COLD, 3, T]
        ph = sh[:, None, None, None] * x3[None]                   # [HOT, 3, NT, T]
        peh = np.concatenate([np.sin(ph), np.cos(ph)], axis=0)    # [2*HOT, 3, NT, T]
        peh = np.ascontiguousarray(peh.transpose(2, 0, 1, 3)).astype(np.float16)
        # lab rows: lab_d[t, r, mp, j] = lemb[point t*T+j, mp*128+r]
        lc = lemb[b0 : b0 + 4].reshape(NPTS, 2, 128)              # [pts, mp, r]
        lc = np.ascontiguousarray(
            lc.reshape(NT, T, 2, 128).transpose(0, 3, 2, 1)
        )                                                         # [NT, 128, 2, T]
        in_maps.append({"bc": bcc, "peh": peh, "lab": lc, **shared})
    return in_maps


def _get_nc():
    if "nc" not in _CACHE:
        _CACHE["nc"] = _build_program()
    return _CACHE["nc"]


def _run_device(in_maps, trace=False, **kw):
    nc = _get_nc()
    return run_bass_kernel_spmd(nc, in_maps, list(range(NCORES)), trace=trace, **kw)


def kernel(point_coord, labels, pc_range, noise, query_pos, label_weight, W1, b1, W2, b2):
    in_maps = _host_prep(
        point_coord, labels, pc_range, noise, label_weight, W1, b1, W2, b2
    )
    res = _run_device(in_maps)

    qp = np.asarray(query_pos, np.float32)
    out = np.empty((G * B, N, 4 * F), np.float32)
    out[:, :, : 2 * F] = qp.reshape(G * B, N, 2 * F)
    for core in range(NCORES):
        q = res.results[core]["q"]                       # [NT, 256, T] f16
        q = q.transpose(1, 0, 2).reshape(2 * F, BPC, N).transpose(1, 2, 0)
        out[4 * core : 4 * core + 4, :, 2 * F :] = q.astype(np.float32)
    return out
